# revision 1
# baseline (speedup 1.0000x reference)
"""FlowNetC correlation (nn_Correlation_27797028340332) on 8 TRN2 NeuronCores.

out[b, dy*21+dx, y, x] = mean_c in1[b,c,y,x] * in2p[b,c,y+2*dy, x+2*dx]
with in2p = zero-pad(in2, 20) and (dy, dx) over a 21x21 stride-2 grid.

Strategy (per core; data-parallel over batch B=8):
  - The per-pixel C=256 dot products are cast as banded Gram matmuls on the
    TensorEngine: for each output row y and vertical shift dy, compute
    G[x, x'] = sum_c in1[c,y,x] * in2p[c,y2,x'] restricted to matching x
    parity (the displacement stride is 2, so only same-parity (x, x') pairs
    are needed).  Two parity Grams of shape [64, 84] are computed per (y,dy)
    as column-tiled fp16 matmuls (even -> PSUM partitions 0:63, odd ->
    64:127), contraction C=256 split into 2 accumulating chunks, dy's
    batched along the moving dim (<= 6*84 = 504 columns, one PSUM bank).
  - The needed output band G[x, x+2k] (k=0..20) is a per-partition diagonal,
    which no engine can extract at line rate, so the parity Grams (4x the
    output elements) are cast to fp16 and dumped contiguously to DRAM; the
    cheap shear is a numpy strided view on the host inside kernel().
"""

import numpy as np

B, C, H, W = 8, 256, 96, 128
PAD = 20
D = 21           # displacements per axis
CH = 2           # contraction chunks of 128
WPAD = W + 2 * PAD   # 168
WE = WPAD // 2       # 84 (per-parity padded width)
XE = W // 2          # 64 (per-parity output columns)
N_CORES = 8
MAX_DY_BATCH = 6     # 6*84 = 504 <= 512 fp32 PSUM-bank limit
ROWBLK = 8


def _valid_dys(y):
    """dy' indices with in-range source row y2 = y + 2*dy' - 20."""
    return [d for d in range(D) if 0 <= y + 2 * d - PAD < H]


def _batches(n):
    """Split n dy's into balanced batches of size <= MAX_DY_BATCH."""
    nb = -(-n // MAX_DY_BATCH)
    base, rem = divmod(n, nb)
    return [base + (1 if i < rem else 0) for i in range(nb)]


def _dump_layout():
    """Per-y (n_dy, element offset) layout of the dump tensor's free dim."""
    offs, off = [], 0
    for y in range(H):
        n = len(_valid_dys(y))
        offs.append((n, off))
        off += n * WE
    return offs, off  # off = total free-dim elements per partition


_NC_CACHE = {}


def _build(reps=1, mode='full', in2_hwdge=True):
    import contextlib

    import concourse.bacc as bacc
    import concourse.tile as tile
    import concourse.tile_rust as tile_rust
    from concourse import mybir

    offs, total = _dump_layout()

    nc = bacc.Bacc("TRN2", target_bir_lowering=False, debug=False)
    in1_d = nc.dram_tensor("in1", [C, H, W], mybir.dt.float32,
                           kind="ExternalInput").ap()
    in2_d = nc.dram_tensor("in2", [C, H, W], mybir.dt.float32,
                           kind="ExternalInput").ap()
    dump_d = nc.dram_tensor("dump", [128, total], mybir.dt.float16,
                            kind="ExternalOutput").ap()

    with tile.TileContext(nc) as tc:
        with tc.tile_pool(name="resident", bufs=1) as res_pool, \
             tc.tile_pool(name="stage", bufs=4) as stage_pool, \
             tc.tile_pool(name="out", bufs=6) as out_pool, \
             tc.tile_pool(name="psum", bufs=8, space="PSUM") as psum_pool, \
             (tc.For_i(0, reps, 1) if reps > 1 else contextlib.nullcontext()):

            # Fully-resident fp16 feature maps; in2 zero-padded along x.
            in1s = res_pool.tile([128, CH, H, W], mybir.dt.float16)
            in2p = res_pool.tile([128, CH, H, WPAD], mybir.dt.float16)
            nc.vector.memset(in2p[:, :, :, 0:PAD], 0.0)
            nc.vector.memset(in2p[:, :, :, W + PAD:WPAD], 0.0)

            def load_block(yb):
                """DMA + fp16-cast rows [yb*8, yb*8+8) of both inputs."""
                y0 = yb * ROWBLK
                s1 = stage_pool.tile([128, CH, ROWBLK, W], mybir.dt.float32,
                                     tag="s1")
                nc.gpsimd.dma_start(
                    s1[:], in1_d[:, y0:y0 + ROWBLK, :].rearrange(
                        "(k p) y x -> p k y x", p=128))
                nc.vector.tensor_copy(in1s[:, :, y0:y0 + ROWBLK, :], s1[:])

                s2 = stage_pool.tile([128, CH, ROWBLK, W], mybir.dt.float32,
                                     tag="s2")
                (nc.scalar if in2_hwdge else nc.gpsimd).dma_start(
                    s2[:], in2_d[:, y0:y0 + ROWBLK, :].rearrange(
                        "(k p) y x -> p k y x", p=128))
                nc.scalar.copy(
                    in2p[:, :, y0:y0 + ROWBLK, PAD:PAD + W], s2[:])

            # Prologue: rows 0..31 cover y=0 (in2 reads reach row 20).
            for yb in range(4):
                load_block(yb)

            def copy_dve(out, in_):
                nc.vector.tensor_copy(out, in_)

            def copy_act(out, in_):
                nc.scalar.copy(out, in_)

            copy_eng = [copy_dve, copy_act]

            for y in range(H):
                # Stay 3-4 blocks ahead of the in2 read frontier (y+20).
                if y % ROWBLK == 0:
                    yb = y // ROWBLK + 4
                    if yb < H // ROWBLK:
                        load_block(yb)

                dys = _valid_dys(y)
                n_dy, off = offs[y]

                stage = out_pool.tile([128, D * WE], mybir.dt.float16,
                                      tag="dumpstage")
                bi = 0
                boff = 0
                for bsz in _batches(n_dy) if mode in ('full', 'nodump') else []:
                    dy0 = dys[bi]
                    y2f = y + 2 * dy0 - PAD
                    # Pad to 512 f32 = one full 2 KiB bank so partition-
                    # sliced matmul outputs stay bank-aligned.
                    ps = psum_pool.tile([128, 512], mybir.dt.float32,
                                        tag="ps")
                    # The 4 matmuls of one tile must stay in program order:
                    # start=True clears has_written for the whole bank, so
                    # the odd-parity group may not begin before the even
                    # group's stop (the scheduler sees no data overlap).
                    prev = None
                    for par in range(2):
                        for ch in range(CH):
                            mm = nc.tensor.matmul(
                                ps[par * XE:(par + 1) * XE, :bsz * WE],
                                in1s[:, ch, y, par::2],
                                in2p[:, ch, y2f:min(H, y2f + 2 * bsz):2,
                                     par::2],
                                start=(ch == 0), stop=(ch == CH - 1))
                            if prev is not None:
                                tile_rust.add_dep_helper(
                                    mm.ins, prev.ins, sync=False,
                                    reason="psum group order")
                            prev = mm
                    # PSUM -> fp16 staging (alternate DVE / ACT)
                    copy_eng[bi % 2](stage[:, boff:boff + bsz * WE],
                                     ps[:, :bsz * WE])
                    bi += bsz
                    boff += bsz * WE

                if mode == 'full':
                    nc.sync.dma_start(dump_d[:, off:off + n_dy * WE],
                                      stage[:, :n_dy * WE])
                elif mode == 'dma':
                    # dump same byte count from a resident tile (no compute)
                    flat = in1s[:, 0, :, :].rearrange("p a b -> p (a b)")
                    nc.gpsimd.dma_start(
                        dump_d[:, off:off + n_dy * WE],
                        flat[:, :n_dy * WE])

    nc.compile()
    return nc, offs, total


def _get_nc():
    if "nc" not in _NC_CACHE:
        _NC_CACHE["nc"] = _build()
    return _NC_CACHE["nc"]


def _assemble(dump, offs):
    """Shear one core's fp16 Gram dump into [441, H, W] fp32."""
    out = np.zeros((D * D, H, W), np.float32)
    ks = np.arange(D)
    for y in range(H):
        n, off = offs[y]
        blk = np.ascontiguousarray(
            dump[:, off:off + n * WE]).astype(np.float32) / np.float32(C)
        blk = blk.reshape(128, n, WE)
        dys = np.array(_valid_dys(y))
        d_idx = (dys[:, None] * D + ks[None, :]).ravel()
        for par in range(2):
            g = blk[par * XE:(par + 1) * XE]          # [64, n, 84]
            s = g.strides
            diag = np.lib.stride_tricks.as_strided(
                g, shape=(n, D, XE), strides=(s[1], s[2], s[0] + s[2]))
            out[d_idx, y, par::2] = diag.reshape(n * D, XE)
    return out


def kernel(input1: np.ndarray, input2: np.ndarray) -> np.ndarray:
    from concourse.bass_utils import run_bass_kernel_spmd

    nc, offs, total = _get_nc()
    in_maps = [
        {"in1": np.ascontiguousarray(input1[b], np.float32),
         "in2": np.ascontiguousarray(input2[b], np.float32)}
        for b in range(N_CORES)
    ]
    res = run_bass_kernel_spmd(nc, in_maps, list(range(N_CORES)))
    out = np.empty((B, D * D, H, W), np.float32)
    for b in range(N_CORES):
        out[b] = _assemble(res.results[b]["dump"], offs)
    return out



# revision 5
# speedup vs baseline: 65.5400x; 65.5400x over previous
"""FlowNetC correlation (nn_Correlation_27797028340332) on 8 TRN2 NeuronCores.

out[b, dy*21+dx, y, x] = mean_c in1[b,c,y,x] * in2p[b,c,y+2*dy, x+2*dx]
with in2p = zero-pad(in2, 20) and (dy, dx) over a 21x21 stride-2 grid.

Strategy (per core; data-parallel over batch B=8):
  - Inputs are cast to fp16 on the host, so the device reads 12.6 MB instead
    of 25.2 MB and needs no on-chip cast.
  - The per-pixel C=256 dot products are banded Gram matmuls, but with in2p
    as the STATIONARY operand: for output row y2 of in2p, parity par, and an
    8-column x-tile t, the stationary is the 28-wide window
    in2p[c, y2, 8t .. 8t+27] (parity coords) and the moving operand is
    in1[c, y, 8t .. 8t+7] for all rows y that pair with y2
    (y = y2 + 20 - 2*dy, ny <= 21 of them, one strided AP).  PSUM gets
    G[x', (y, x)] = sum_c in2p[c,y2,x'] * in1[c,y,x], a [28, ny*8] block in
    which the needed band dx = x' - x in [0, 20] always lies inside the
    window.  Redundancy is 32/21 (vs 4x for the 84-wide Gram).
  - The 16 blocks per y2 (2 parities x 8 tiles) are packed 4-per-PSUM-bank
    in the 4 column groups (partitions 32g..32g+27) and run as CONCURRENT
    column-tiled matmuls (tile_position via out base partition): has_written
    zero regions are partition-scoped, so the 4 accumulation groups in one
    bank are independent.  One [128, ny*8] copy per bank evacuates all 4.
  - Band extraction (diagonal shear) is numpy on the host; HW dumps the
    fp16 blocks contiguously (14.7 MB vs 38.6 MB for the 84-wide layout).
"""

import numpy as np

B, C, H, W = 8, 256, 96, 128
PAD = 20
D = 21           # displacements per axis
CH = 2           # contraction chunks of 128
WPAD = W + 2 * PAD   # 168
WE = WPAD // 2       # 84 (per-parity padded width)
XE = W // 2          # 64 (per-parity output columns)
N_CORES = 8
T = 8            # x-tile width (parity coords); 8 tiles cover XE=64
WIN = T + PAD    # 28-wide stationary window per tile
ROWBLK = 8


def _y_range(y2):
    """Valid in1 rows pairing with in2 row y2: y = y2 + 20 - 2*dy, dy 0..20,
    0 <= y < H.  Returns (y_lo, ny); rows are y_lo, y_lo+2, ..."""
    y_lo = y2 - PAD if y2 >= PAD else y2 % 2
    y_hi = y2 + PAD if y2 + PAD < H else H - 1 - ((H - 1 - y2) % 2)
    return y_lo, (y_hi - y_lo) // 2 + 1


def _dump_layout():
    """Per-y2 (ny, element offset) layout of the dump tensor's free dim."""
    offs, off = [], 0
    for y2 in range(H):
        _, ny = _y_range(y2)
        offs.append((ny, off))
        off += 4 * ny * T  # 4 rounds (banks) x [ny, T] per partition
    return offs, off


_NC_CACHE = {}


def _build(reps=1, mode='full'):
    import contextlib

    import concourse.bacc as bacc
    import concourse.tile as tile
    from concourse import mybir

    offs, total = _dump_layout()

    nc = bacc.Bacc("TRN2", target_bir_lowering=False, debug=False)
    in1_d = nc.dram_tensor("in1", [C, H, W], mybir.dt.float16,
                           kind="ExternalInput").ap()
    in2_d = nc.dram_tensor("in2", [C, H, W], mybir.dt.float16,
                           kind="ExternalInput").ap()
    dump_d = nc.dram_tensor("dump", [128, total], mybir.dt.float16,
                            kind="ExternalOutput").ap()

    with tile.TileContext(nc) as tc:
        with tc.tile_pool(name="resident", bufs=1) as res_pool, \
             tc.tile_pool(name="out", bufs=6) as out_pool, \
             tc.tile_pool(name="psum", bufs=8, space="PSUM") as psum_pool:

            # Fully-resident fp16 feature maps; in2 zero-padded along x.
            # Pad memsets stay outside the reps loop (never overwritten).
            in1s = res_pool.tile([128, CH, H, W], mybir.dt.float16)
            in2p = res_pool.tile([128, CH, H, WPAD], mybir.dt.float16)
            nc.vector.memset(in2p[:, :, :, 0:PAD], 0.0)
            nc.vector.memset(in2p[:, :, :, W + PAD:WPAD], 0.0)

            with (tc.For_i(0, reps, 1) if reps > 1 else
                  contextlib.nullcontext()):

                def load_block(yb):
                    """DMA rows [yb*8, yb*8+8) of both inputs (fp16)."""
                    y0 = yb * ROWBLK
                    nc.gpsimd.dma_start(
                        in1s[:, :, y0:y0 + ROWBLK, :],
                        in1_d[:, y0:y0 + ROWBLK, :].rearrange(
                            "(k p) y x -> p k y x", p=128))
                    for k in range(CH):
                        # per-chunk: the padded dest makes the fused AP
                        # 4-dim, which the DMA balancer rejects
                        nc.scalar.dma_start(
                            in2p[:, k, y0:y0 + ROWBLK, PAD:PAD + W],
                            in2_d[128 * k:128 * (k + 1),
                                  y0:y0 + ROWBLK, :].rearrange(
                                "(k p) y x -> p k y x", p=128))

                # Prologue: rows 0..31 cover y2=0 (in1 reads reach row 20).
                for yb in range(4):
                    load_block(yb)

                def copy_dve(out, in_):
                    nc.vector.tensor_copy(out, in_)

                def copy_act(out, in_):
                    nc.scalar.copy(out, in_)

                copy_eng = [copy_dve, copy_act]

                for y2 in range(H):
                    if y2 % ROWBLK == 0:
                        yb = y2 // ROWBLK + 4
                        if yb < H // ROWBLK:
                            load_block(yb)

                    y_lo, ny = _y_range(y2)
                    _, off = offs[y2]
                    nf = ny * T  # free extent per block

                    stage = out_pool.tile([128, 4 * D * T], mybir.dt.float16,
                                          tag="stage")
                    for r in range(4):
                        ps = psum_pool.tile([128, 512], mybir.dt.float32,
                                            tag="ps")
                        for g in range(4):
                            b = 4 * r + g
                            par, til = b // 8, b % 8
                            x0 = par + 2 * T * til
                            for ch in range(CH):
                                # explicit tile_position: auto-infer rejects
                                # out base partition 96 (bass_types quirk)
                                nc.tensor.matmul(
                                    ps[32 * g:32 * g + WIN, :nf],
                                    in2p[:, ch, y2,
                                         x0:x0 + 2 * WIN - 1:2],
                                    in1s[:, ch, y_lo:y_lo + 2 * ny - 1:2,
                                         x0:x0 + 2 * T - 1:2],
                                    start=(ch == 0), stop=(ch == CH - 1),
                                    tile_position=(0, 32 * g))
                        # PSUM -> fp16 staging (alternate DVE / ACT)
                        copy_eng[r % 2](stage[:, r * nf:(r + 1) * nf],
                                        ps[:, :nf])

                    if mode == 'full':
                        nc.sync.dma_start(dump_d[:, off:off + 4 * nf],
                                          stage[:, :4 * nf])

    nc.compile()
    return nc, offs, total


def _get_nc():
    if "nc" not in _NC_CACHE:
        _NC_CACHE["nc"] = _build()
    return _NC_CACHE["nc"]


def _core_in_map(in1_f32, in2_f32):
    """Host-side fp16 cast; the device reads fp16 directly."""
    return {"in1": np.ascontiguousarray(in1_f32, np.float16),
            "in2": np.ascontiguousarray(in2_f32, np.float16)}


def _assemble(dump, offs):
    """Shear one core's fp16 block dump into [441, H, W] fp32."""
    out = np.zeros((D * D, H, W), np.float32)
    dxs = np.arange(D)
    for y2 in range(H):
        ny, off = offs[y2]
        y_lo = y2 - PAD if y2 >= PAD else y2 % 2
        dy0 = (y2 - y_lo + PAD) // 2
        nf = ny * T
        blk = np.ascontiguousarray(
            dump[:, off:off + 4 * nf]).astype(np.float32) / np.float32(C)
        # [g, w, r, yi, j]
        blk = blk.reshape(4, 32, 4, ny, T)
        yis = np.arange(ny)
        d_idx = (dy0 - yis)[:, None] * D + dxs[None, :]   # [ny, 21]
        ys = y_lo + 2 * yis                               # [ny]
        for r in range(4):
            for g in range(4):
                b = 4 * r + g
                par, til = b // 8, b % 8
                v = blk[g, :, r]                          # [32, ny, T]
                s = v.strides
                # diag[yi, dx, j] = v[j + dx, yi, j]
                diag = np.lib.stride_tricks.as_strided(
                    v, shape=(ny, D, T), strides=(s[1], s[0], s[0] + s[2]))
                xs = par + 2 * (T * til + np.arange(T))   # [T]
                out[d_idx[:, :, None], ys[:, None, None],
                    xs[None, None, :]] = diag
    return out


def kernel(input1: np.ndarray, input2: np.ndarray) -> np.ndarray:
    from concourse.bass_utils import run_bass_kernel_spmd

    nc, offs, total = _get_nc()
    in_maps = [_core_in_map(input1[b], input2[b]) for b in range(N_CORES)]
    res = run_bass_kernel_spmd(nc, in_maps, list(range(N_CORES)))
    out = np.empty((B, D * D, H, W), np.float32)
    for b in range(N_CORES):
        out[b] = _assemble(res.results[b]["dump"], offs)
    return out
